# revision 1
# baseline (speedup 1.0000x reference)
"""Trainium2 Bass kernel for nn_Attention (B=1, C=64, 12x12x12 spatial, 32 heads, head_dim=2).

Sharding: 32 heads split across 8 cores (4 heads/core). Each core computes
qkv projection for its heads, head-local attention (flash-style: S^T chunks
-> exp on ScalarE -> U/Z accumulation via matmul with V'=[V,1]), divides,
then applies its slice of w_proj rows to produce a partial output summed on
the host (tensor-parallel unshard) with bias/8 folded per core.

Uses bacc.Bacc (not plain Bass): its compile() runs
move_matmul_waits_to_ldweights + generate_event_semaphores, which the
TRN2 one-wait-per-instruction ISA constraint requires for Tile kernels.

Scheduling notes: Tile's static scheduler keeps per-engine creation order,
so overlap is structured by emission order — qkv for the second query tile
is emitted inside the first tile's key loop (PE is idle there; ScalarE exp
is the bottleneck), and the first tile's divide/proj are emitted before the
second tile's loop so they run under it.

Self-contained: hardcodes all shapes.
"""

import numpy as np
import ml_dtypes

import concourse.bass as bass
import concourse.bacc as bacc
import concourse.mybir as mybir
from concourse import tile
from concourse.bass_utils import run_bass_kernel_spmd

C = 64
N = 1728  # 12*12*12
NCORES = 8
HLOC = 4          # heads per core
SCALE = float(2.0 ** -0.5)

# key chunks: 13x128 + 64
KCS = [(i * 128, 128) for i in range(13)] + [(1664, 64)]
NKC = len(KCS)
# query tiles: big first tile, small second so the un-overlappable tail
# (reciprocal is FD-bound at 8 cyc/elem) is short
QTS = [(0, 1024), (1024, 704)]
# token chunks for proj; chunks 0..8 lie fully inside query tile 0
TCS = [(i * 108, 108) for i in range(16)]
TC_SPLIT = 9

F32 = mybir.dt.float32
BF16 = mybir.dt.bfloat16


def _sub_mms(qn):
    out = []
    o = 0
    while o < qn:
        n = min(512, qn - o)
        out.append((o, n))
        o += n
    return out


def build_nc():
    nc = bacc.Bacc(None)

    x2 = nc.declare_dram_parameter("x2", [C, N], BF16, isOutput=False)
    wq = nc.declare_dram_parameter("wq", [C, 2 * HLOC], BF16, isOutput=False)
    wk = nc.declare_dram_parameter("wk", [C, 2 * HLOC], BF16, isOutput=False)
    wv = nc.declare_dram_parameter("wv", [C, 2 * HLOC], BF16, isOutput=False)
    wp = nc.declare_dram_parameter("wp", [2 * HLOC + 1, C], F32, isOutput=False)
    y = nc.declare_dram_parameter("y", [N, C], F32, isOutput=True)

    with tile.TileContext(nc) as tc:
        with (
            tc.tile_pool(name="const", bufs=1) as cpool,
            tc.tile_pool(name="epool", bufs=5) as epool,
            tc.tile_pool(name="upool", bufs=2) as upool,
            tc.tile_pool(name="ps_s", bufs=3, space=bass.MemorySpace.PSUM) as ps_s,
            tc.tile_pool(name="ps_u", bufs=1, space=bass.MemorySpace.PSUM) as ps_u,
        ):
            x_st = cpool.tile([C, N], BF16, name="x_st")
            x_sb = cpool.tile([C, N], BF16, name="x_sb")
            wq_st = cpool.tile([C, 2 * HLOC], BF16, name="wq_st")
            wq_sb = cpool.tile([C, 2 * HLOC], BF16, name="wq_sb")
            wk_st = cpool.tile([C, 2 * HLOC], BF16, name="wk_st")
            wk_sb = cpool.tile([C, 2 * HLOC], BF16, name="wk_sb")
            wv_st = cpool.tile([C, 2 * HLOC], BF16, name="wv_st")
            wv_sb = cpool.tile([C, 2 * HLOC], BF16, name="wv_sb")
            wp_st = cpool.tile([2 * HLOC + 1, C], F32, name="wp_st")
            wp_sb = cpool.tile([2 * HLOC + 1, C], F32, name="wp_sb")
            qT = cpool.tile([128, N], BF16, name="qT")
            kT = cpool.tile([128, N], BF16, name="kT")
            vp = cpool.tile([128, NKC * 3 * HLOC], BF16, name="vp")
            ot = cpool.tile([2 * HLOC + 1, N], F32, name="ot")
            ybig = cpool.tile([128, len(TCS) * C], F32, name="ybig")
            ybv = ybig[:].rearrange("p (t c) -> p t c", c=C)

            # x DMA first (it gates everything); stage through one DVE copy
            # each so consumers wait on a single engine semaphore instead of
            # one per DMA queue.
            nc.sync.dma_start(out=x_st[:], in_=x2[:])
            nc.sync.dma_start(out=wv_st[:], in_=wv[:])
            nc.sync.dma_start(out=wq_st[:], in_=wq[:])
            nc.sync.dma_start(out=wk_st[:], in_=wk[:])
            nc.sync.dma_start(out=wp_st[:], in_=wp[:])
            nc.vector.tensor_copy(x_sb[:, 0:1024], x_st[:, 0:1024])
            nc.vector.tensor_copy(x_sb[:, 1024:N], x_st[:, 1024:N])
            nc.vector.tensor_copy(wv_sb[:], wv_st[:])
            nc.vector.tensor_copy(wq_sb[:], wq_st[:])
            nc.vector.tensor_copy(wk_sb[:], wk_st[:])
            nc.vector.tensor_copy(wp_sb[:], wp_st[:])

            # ones row for proj bias (rows 0..7 overwritten by attention out)
            nc.gpsimd.memset(ot[:, :], 1.0)
            # ones column per head in V' ([128, kc, h, 3], col 2 = 1.0)
            vp_v = vp[:].rearrange("p (a b c) -> p a b c", b=HLOC, c=3)
            nc.gpsimd.memset(vp_v[:, :, :, 2:3], 1.0)

            # ---- V': all 14 key chunks' V rows packed into ONE psum tile
            # (emitted as pre_u of kc0 — only U matmuls need it) ----
            def emit_vprime():
                psv = ps_s.tile([128, 1024], F32, tag="s", name="ps_v")
                for kc, (ko, kn) in enumerate(KCS):
                    nc.tensor.matmul(
                        psv[:kn, 8 * kc : 8 * kc + 2 * HLOC],
                        x_sb[:, ko : ko + kn],
                        wv_sb[:, 0 : 2 * HLOC],
                        start=True, stop=True,
                    )
                vsrc = psv[:, 0 : 8 * NKC].rearrange(
                    "p (kc h d) -> p kc h d", h=HLOC, d=2
                )
                nc.vector.tensor_copy(vp_v[:, :, :, 0:2], vsrc)

            def qkv_tile(w_sb, dst, off, qn, heads=range(HLOC)):
                """Per-head matmuls (rows at partitions 32h) + per-head copy."""
                ps = ps_s.tile([128, 1024], F32, tag="s", name="ps_qkv")
                for h in heads:
                    for (o, n_) in _sub_mms(qn):
                        nc.tensor.matmul(
                            ps[32 * h : 32 * h + 2, o : o + n_],
                            w_sb[:, 2 * h : 2 * h + 2],
                            x_sb[:, off + o : off + o + n_],
                            start=True, stop=True,
                            tile_position=(0, 32 * h),
                        )
                    nc.vector.tensor_copy(
                        dst[32 * h : 32 * h + 2, off : off + qn],
                        ps[32 * h : 32 * h + 2, :qn],
                    )

            # q half 0 / first k cols are emitted per-head just before each
            # head's first S matmul (pre_s of kc0) so exp h0 starts ASAP
            def pre_s0(kc, h):
                if kc == 0:
                    qkv_tile(wq_sb, qT, 0, 1024, heads=[h])
                    qkv_tile(wk_sb, kT, 0, 512, heads=[h])

            def pre_u0(kc):
                if kc == 0:
                    emit_vprime()

            def divide_and_store(pu, qo, qn, last=False):
                """O^T rows 2h+d of `ot` <- U rows / Z row (per head)."""
                if last:
                    # final tile: read PSUM directly, no next user of the slot
                    usrc = pu[:, :qn]
                else:
                    u_sb = upool.tile([128, 1024], F32, tag="u_sb", name="u_sb")
                    nc.vector.tensor_copy(u_sb[:, :qn], pu[:, :qn])
                    usrc = u_sb[:, :qn]
                zrec = upool.tile([128, 1024], F32, tag="zrec", name="zrec")
                nc.vector.reciprocal(zrec[:, :qn], usrc)
                zz = upool.tile([128, 1024], F32, tag="zz", name="zz")
                zzv_ = zz[:, :qn].rearrange("(h g) f -> h g f", g=32)
                zrv_ = zrec[:, :qn].rearrange("(h g) f -> h g f", g=32)
                nc.sync.dma_start(out=zzv_[:, 0, :], in_=zrv_[:, 2, :])
                nc.gpsimd.dma_start(out=zzv_[:, 1, :], in_=zrv_[:, 2, :])
                osp = upool.tile([128, 1024], F32, tag="osp", name="osp")
                nc.vector.tensor_mul(osp[:, :qn], usrc, zz[:, :qn])
                ospv = osp[:, :qn].rearrange("(h g) f -> h g f", g=32)
                otv = ot[0 : 2 * HLOC, qo : qo + qn].rearrange("(h g) f -> h g f", g=2)
                nc.sync.dma_start(out=otv[:, 0, :], in_=ospv[:, 0, :])
                nc.gpsimd.dma_start(out=otv[:, 1, :], in_=ospv[:, 1, :])

            def proj_chunks(ts_):
                for t in ts_:
                    to, tn = TCS[t]
                    py = ps_s.tile([128, 1024], F32, tag="s", name="py")
                    nc.tensor.matmul(
                        py[:tn, 0:C], ot[:, to : to + tn], wp_sb[:],
                        start=True, stop=True,
                    )
                    nc.vector.tensor_copy(ybv[:tn, t, :], py[:tn, 0:C])

            def main_loop(qo, qn, boundary_work, pre_s=None, pre_u=None):
                pu = ps_u.tile([128, 1024], F32, tag="pu", name="pu")
                for kc, (ko, kn) in enumerate(KCS):
                    es = []
                    for h in range(HLOC):
                        if pre_s is not None:
                            pre_s(kc, h)
                        ps = ps_s.tile([128, 1024], F32, tag="s", name="ps_att")
                        for (o, n_) in _sub_mms(qn):
                            nc.tensor.matmul(
                                ps[:kn, o : o + n_],
                                kT[32 * h : 32 * h + 2, ko : ko + kn],
                                qT[32 * h : 32 * h + 2, qo + o : qo + o + n_],
                                start=True, stop=True,
                                tile_position=(32 * h, 0),
                            )
                        e = epool.tile([128, 1024], BF16, tag="e", name="e")
                        nc.scalar.activation(
                            e[:kn, :qn], ps[:kn, :qn],
                            mybir.ActivationFunctionType.Exp, scale=SCALE,
                        )
                        es.append(e)
                    if pre_u is not None:
                        pre_u(kc)
                    for h in range(HLOC):
                        for (o, n_) in _sub_mms(qn):
                            nc.tensor.matmul(
                                pu[32 * h : 32 * h + 3, o : o + n_],
                                vp_v[:kn, kc, h, :],
                                es[h][:kn, o : o + n_],
                                start=(kc == 0), stop=(kc == NKC - 1),
                                tile_position=(0, 32 * h),
                            )
                    work = boundary_work.get(kc)
                    if work:
                        work()
                return pu

            # qt0 loop: remaining qkv emitted at key-loop boundaries, one
            # small piece per boundary (PE slack under the ACT-bound loop)
            bw0 = {
                0: lambda: qkv_tile(wk_sb, kT, 512, 512),
                1: lambda: qkv_tile(wq_sb, qT, 1024, 704, heads=[0, 1]),
                2: lambda: qkv_tile(wq_sb, qT, 1024, 704, heads=[2, 3]),
                3: lambda: qkv_tile(wk_sb, kT, 1024, 704, heads=[0, 1]),
                4: lambda: qkv_tile(wk_sb, kT, 1024, 704, heads=[2, 3]),
            }
            pu0 = main_loop(0, 1024, bw0, pre_s=pre_s0, pre_u=pre_u0)
            divide_and_store(pu0, 0, 1024)

            # qt1 loop: qt0's proj + first y DMA emitted at late boundaries
            # (after qt0's divide chain has drained on DVE/DMA)
            def y_dma0():
                yv0 = y[0 : TC_SPLIT * 108, :].rearrange("(t i) c -> i t c", i=108)
                nc.sync.dma_start(out=yv0, in_=ybv[:108, 0:TC_SPLIT, :])

            bw1 = {kc: (lambda t=kc - 3: proj_chunks([t])) for kc in range(3, 12)}
            bw1[12] = y_dma0
            pu1 = main_loop(1024, 704, bw1)
            divide_and_store(pu1, 1024, 704, last=True)
            proj_chunks(range(TC_SPLIT, len(TCS)))
            yv1 = y[TC_SPLIT * 108 :, :].rearrange("(t i) c -> i t c", i=108)
            nc.sync.dma_start(out=yv1, in_=ybv[:108, TC_SPLIT:, :])

    return nc


_NC = None


def _get_nc():
    global _NC
    if _NC is None:
        _NC = build_nc()
        _NC.finalize()
    return _NC


def make_in_maps(x, w_qkv, w_proj, b_proj):
    x2 = np.ascontiguousarray(x.reshape(C, N)).astype(ml_dtypes.bfloat16)
    in_maps = []
    for c in range(NCORES):
        sl = slice(8 * c, 8 * c + 8)
        wq = np.ascontiguousarray(w_qkv[sl, :].T).astype(ml_dtypes.bfloat16)
        wk = np.ascontiguousarray(w_qkv[64 + 8 * c : 64 + 8 * c + 8, :].T).astype(
            ml_dtypes.bfloat16
        )
        wv = np.ascontiguousarray(w_qkv[128 + 8 * c : 128 + 8 * c + 8, :].T).astype(
            ml_dtypes.bfloat16
        )
        wp = np.concatenate(
            [w_proj[:, sl].T, (b_proj / NCORES)[None, :]], axis=0
        ).astype(np.float32)
        in_maps.append(
            {"x2": x2, "wq": wq, "wk": wk, "wv": wv, "wp": np.ascontiguousarray(wp)}
        )
    return in_maps


def run(x, w_qkv, w_proj, b_proj, trace=False, **kw):
    nc = _get_nc()
    in_maps = make_in_maps(x, w_qkv, w_proj, b_proj)
    res = run_bass_kernel_spmd(
        nc, in_maps, core_ids=list(range(NCORES)), trace=trace, **kw
    )
    y = np.zeros((N, C), np.float32)
    for r in res.results:
        y += r["y"]
    return y.reshape(1, 12, 12, 12, C), res


def kernel(x, w_qkv, w_proj, b_proj):
    out, _ = run(
        np.asarray(x), np.asarray(w_qkv), np.asarray(w_proj), np.asarray(b_proj)
    )
    return out

